# revision 18
# baseline (speedup 1.0000x reference)
"""Multi-head attention (16 heads, d_model=1024, bs=2, qlen=2048) on 8 trn2 cores.

Sharding: core c -> batch b = c//4, head-group r = c%4 (heads 4r..4r+3, i.e.
dims 256r..256r+256 of the head axis).  Each core projects q/k/v only for its
own 4 heads (Megatron column split), runs scores/softmax/AV for those heads,
then per-unit AllGathers of the per-core context within each batch group of 4
cores feed the row-split output projection (each core computes its own 256
output columns; no reduction needed).

v2 layout/schedule:
 - Inputs are host-packed so every DMA moves multi-KB rows: x arrives as a
   [128, 8, 2048] tile in 8 x 512KB chunk transfers; each weight is a single
   [128, 8, 256] tile.
 - k projection is kc-major and streams behind the x DMA: all 8 (n,m)
   accumulators live in PSUM at once, so chunk kc is consumed right after it
   lands and kT2 is ready just after the last x chunk (~+15us).
 - Attention units are (pair, query-slice); the last pair-1 slice is split in
   half so the final AllGather is 64KB and the tail is short.
 - Scores for the two heads of a pair are two concurrent row-tiled K=64
   matmuls sharing one streamed q tile into one [128,1024] PSUM tile; exp on
   the scalar engine is the attention-phase pacer (~1.07us/chunk); V is stored
   interleaved with ones columns so one M=128 matmul per head accumulates both
   context rows and softmax denominators.
 - Trailing work (v projection in unit 0, q projection for later tiles, the
   output projections) is dripped between the score and AV groups; each
   accumulation's pieces stay within one unit so the "op" PSUM rotation never
   crosses a normalize boundary.
 - normalize first copies both ctx/denominator PSUM banks to SBUF so the next
   unit's AV can reclaim them early; the denominator swap+broadcast is two
   concurrent row-tiled selector matmuls.
 - 1/sqrt(d) and q_b folded into q_w/q_b on the host; v_b folded into o_b.
"""

import functools
import os
import sys

import numpy as np

for _p in ("/opt/trn_rl_repo", "/root/.axon_site/_ro/trn_rl_repo"):
    if os.path.isdir(_p) and _p not in sys.path:
        sys.path.append(_p)

import ml_dtypes

from concourse import bacc, bass, mybir, tile
from concourse.bass_utils import run_bass_kernel_spmd

BF16 = ml_dtypes.bfloat16
FP32 = mybir.dt.float32
BF16_DT = mybir.dt.bfloat16

N_CORES = 8
BS = 2
L = 2048  # sequence length
D = 1024  # model dim
DH = 64  # head dim
OWN = 256  # head dims per core (4 heads)
KC_D = 8  # 1024 / 128 contraction chunks for projections
KT = 16  # 2048 / 128 key-token chunks
WARMUP_MM = 10

LAST_EXEC_NS = None
LAST_RESULTS = None

# attention units: (pair, q0, q1).  The final pair-1 tile is split so the
# last AllGather is half-size and the tail after the last AV is short.
UNITS = [
    (0, 0, 512),  # u0
    (1, 0, 512),  # u1
    (0, 512, 1024),  # u2
    (1, 512, 1024),  # u3
    (0, 1024, 1536),  # u4
    (1, 1024, 1536),  # u5
    (0, 1536, 2048),  # u6
    (1, 1536, 2048),  # u7
]


def _build_nc(apply_mask: bool):
    nc = bacc.Bacc(None, num_devices=N_CORES)

    xq = nc.dram_tensor("xq", [128, KC_D, L], BF16_DT, kind="ExternalInput")
    wk = nc.dram_tensor("wk", [128, KC_D, OWN], BF16_DT, kind="ExternalInput")
    wqvo = nc.dram_tensor(
        "wqvo", [128, 3 * KC_D, OWN], BF16_DT, kind="ExternalInput"
    )
    qb2 = nc.dram_tensor("qb2", [128, 2], FP32, kind="ExternalInput")
    kb2 = nc.dram_tensor("kb2", [128, 2], FP32, kind="ExternalInput")
    ob2 = nc.dram_tensor("ob2", [128, 2], FP32, kind="ExternalInput")
    mask01 = nc.dram_tensor("mask01", [128, KT], FP32, kind="ExternalInput")
    outT = nc.dram_tensor("outT", [OWN, L], BF16_DT, kind="ExternalOutput")

    Exp = mybir.ActivationFunctionType.Exp
    GRPS = [[0, 1, 2, 3], [4, 5, 6, 7]]

    with tile.TileContext(nc) as tc:
        with (
            tc.tile_pool(name="const", bufs=1) as const,
            tc.tile_pool(name="work", bufs=2) as work,
            tc.tile_pool(name="ps", bufs=1, space="PSUM") as ps,
            tc.tile_pool(name="dram", bufs=1, space="DRAM") as dram,
        ):
            # ---- stage inputs: few, large transfers; earliest consumers
            # first.  wk + the first x chunks gate the streaming k projection.
            wk_sb = const.tile([128, KC_D, OWN], BF16_DT, tag="wk", name="wk_sb")
            nc.scalar.dma_start(wk_sb, wk[:, :, :])

            # Warm the collective path at t~0: gather garbage (values
            # unused) so the trigger carries no input dependency, twice to
            # absorb the ncfw cold start before the first real gather.
            for wi in range(2):
                wag_in = dram.tile([128, 2], FP32, tag=f"wagi{wi}", name=f"wagi{wi}")
                wag_out = dram.tile([512, 2], FP32, tag=f"wago{wi}", name=f"wago{wi}")
                nc.gpsimd.collective_compute(
                    "AllGather",
                    mybir.AluOpType.bypass,
                    replica_groups=GRPS,
                    ins=[wag_in.opt()],
                    outs=[wag_out.opt()],
                )


            x_sb = const.tile([128, KC_D, L], BF16_DT, tag="x", name="x_sb")
            nc.sync.dma_start(x_sb[:, 0:2, :], xq[:, 0:2, :])

            def load_small(dram_t, nm, cols):
                t = const.tile([128, cols], FP32, tag=nm, name=f"{nm}_sb")
                nc.sync.dma_start(t, dram_t[:, :])
                return t

            kb_sb = load_small(kb2, "kb", 2)
            qb_sb = load_small(qb2, "qb", 2)
            ob_sb = load_small(ob2, "ob", 2)
            mask_sb = load_small(mask01, "mask", KT) if apply_mask else None

            for i in range(1, 4):
                eng = nc.sync if i % 2 == 0 else nc.scalar
                eng.dma_start(x_sb[:, 2 * i : 2 * i + 2, :], xq[:, 2 * i : 2 * i + 2, :])

            wqvo_sb = const.tile(
                [128, 3 * KC_D, OWN], BF16_DT, tag="wqvo", name="wqvo_sb"
            )
            nc.sync.dma_start(wqvo_sb, wqvo[:, :, :])


            ones_sb = const.tile([128, DH], BF16_DT, tag="ones", name="ones_sb")
            nc.vector.memset(ones_sb, 1.0)

            # selector for the denominator broadcast: after the unit's ctx and
            # denominator banks are copied to SBUF (csA holds [ctx_h0; den_h0],
            # csB holds [den_h1; ctx_h1]), two concurrent row-tiled matmuls
            # build rb = [1-src den_h0 on partitions 0-63; den_h1 on 64-127]:
            #   rb  = selT[64:128]^T @ csA[64:128]   (-> partitions 0:64)
            #   rb += selT[0:64]^T   @ csB[0:64]     (-> partitions 64:128)
            selT = const.tile([128, 128], BF16_DT, tag="selT", name="selT")
            nc.vector.memset(selT, 0.0)
            nc.vector.memset(selT[64:65, 0:64], 1.0)
            nc.vector.memset(selT[0:1, 64:128], 1.0)

            # Warm the PE clock gate while the first inputs stream in.
            wsc = const.tile([128, 512], BF16_DT, tag="wsc", name="wsc")
            nc.vector.memset(wsc, 0.0)
            for _ in range(WARMUP_MM):
                wps = ps.tile([128, 512], FP32, tag="op", bufs=2, name="wps")
                nc.tensor.matmul(wps[0:64, :], lhsT=ones_sb[:, 0:64], rhs=wsc)

            # ---- streaming k projection (kc-major, 8 PSUM accumulators).
            # kT2[p] rows 0-63 = head 2p dims, rows 64-127 = head 2p+1.
            kT2 = [
                const.tile([128, L], BF16_DT, tag=f"kT{p}", name=f"kT2_{p}")
                for p in range(2)
            ]
            sc_t0 = ps.tile([128, 1024], FP32, tag="sc", bufs=2, name="kb_sc0")
            sc_t1 = ps.tile([128, 1024], FP32, tag="sc", bufs=2, name="kb_sc1")
            op_t0 = ps.tile([128, 512], FP32, tag="op", bufs=2, name="kb_op0")
            op_t1 = ps.tile([128, 512], FP32, tag="op", bufs=2, name="kb_op1")
            ctx_t0 = ps.tile([128, 512], FP32, tag="ctx", bufs=1, name="kb_ctx")
            sums_t0 = ps.tile([128, 512], FP32, tag="sums", bufs=1, name="kb_sums")
            kbank = {
                (0, 0): sc_t0[:, 0:512],
                (0, 1): sc_t0[:, 512:1024],
                (1, 0): sc_t1[:, 0:512],
                (1, 1): sc_t1[:, 512:1024],
                (2, 0): op_t0,
                (2, 1): op_t1,
                (3, 0): ctx_t0,
                (3, 1): sums_t0,
            }
            for kc in range(KC_D):
                for m in range(2):
                    for n in range(4):
                        nc.tensor.matmul(
                            kbank[(n, m)],
                            lhsT=wk_sb[:, kc, m * 128 : (m + 1) * 128],
                            rhs=x_sb[:, kc, n * 512 : (n + 1) * 512],
                            start=(kc == 0),
                            stop=(kc == KC_D - 1),
                        )
            for n in range(4):
                for m in range(2):
                    nc.vector.tensor_scalar_add(
                        kT2[m][:, n * 512 : (n + 1) * 512],
                        kbank[(n, m)],
                        kb_sb[:, m : m + 1],
                    )

            # ---- q projection for one 512-token tile (2 m x 8 kc). ----
            qT_sb = [
                const.tile([128, L], BF16_DT, tag=f"qT{p}", name=f"qT_sb{p}")
                for p in range(2)
            ]

            def qproj_pieces(q0, q1):
                nsl = slice(q0, q1)
                state = {}

                def piece(m, lo, hi):
                    def fn():
                        if lo == 0:
                            state[m] = ps.tile(
                                [128, 512], FP32, tag="op", bufs=2, name="projq_ps"
                            )
                        pp = state[m]
                        for kc in range(lo, hi):
                            nc.tensor.matmul(
                                pp[:, 0 : q1 - q0],
                                lhsT=wqvo_sb[:, kc, m * 128 : (m + 1) * 128],
                                rhs=x_sb[:, kc, nsl],
                                start=(kc == 0),
                                stop=(kc == KC_D - 1),
                            )
                        if hi == KC_D:
                            nc.vector.tensor_scalar_add(
                                qT_sb[m][:, nsl],
                                pp[:, 0 : q1 - q0],
                                qb_sb[:, m : m + 1],
                            )
                    return fn

                return [piece(m, lo, lo + 2) for m in (0, 1) for lo in (0, 2, 4, 6)]

            def qproj_pieces2(m):
                # q projection for tiles 1 and 2 together: each weight chunk
                # stationary is loaded once and streams both query tiles.
                state = {}

                def piece(lo, hi):
                    def fn():
                        if lo == 0:
                            state["a"] = ps.tile(
                                [128, 512], FP32, tag="op", bufs=2, name="projq_a"
                            )
                            state["b"] = ps.tile(
                                [128, 512], FP32, tag="op", bufs=2, name="projq_b"
                            )
                        for kc in range(lo, hi):
                            for t, nm in ((512, "a"), (1024, "b")):
                                nc.tensor.matmul(
                                    state[nm],
                                    lhsT=wqvo_sb[:, kc, m * 128 : (m + 1) * 128],
                                    rhs=x_sb[:, kc, t : t + 512],
                                    start=(kc == 0),
                                    stop=(kc == KC_D - 1),
                                )
                        if hi == KC_D:
                            nc.vector.tensor_scalar_add(
                                qT_sb[m][:, 512:1024], state["a"], qb_sb[:, m : m + 1]
                            )
                            nc.vector.tensor_scalar_add(
                                qT_sb[m][:, 1024:1536], state["b"], qb_sb[:, m : m + 1]
                            )
                    return fn

                return [piece(lo, lo + 2) for lo in (0, 2, 4, 6)]

            # ---- v projection (tokens on partitions; no bias), stored
            # interleaved with ones columns: per pair block of 256 cols:
            # [v_h0 | ones | ones | v_h1]. ----
            v_sb = [
                const.tile([128, 512], BF16_DT, tag=f"v{t}", name=f"v_sb{t}")
                for t in range(KT)
            ]

            def vproj_pieces(t):
                state = {}

                def piece(lo, hi):
                    def fn():
                        if lo == 0:
                            nc.vector.memset(v_sb[t][:, 64:192], 1.0)
                            nc.vector.memset(v_sb[t][:, 320:448], 1.0)
                            state["pv"] = ps.tile(
                                [128, 512], FP32, tag="op", bufs=2, name="v_ps"
                            )
                        pv = state["pv"]
                        for kc in range(lo, hi):
                            nc.tensor.matmul(
                                pv[:, 0:OWN],
                                lhsT=x_sb[:, kc, t * 128 : (t + 1) * 128],
                                rhs=wqvo_sb[:, KC_D + kc, :],
                                start=(kc == 0),
                                stop=(kc == KC_D - 1),
                            )
                        if hi == KC_D:
                            nc.vector.tensor_copy(v_sb[t][:, 0:64], pv[:, 0:64])
                            nc.vector.tensor_copy(v_sb[t][:, 192:256], pv[:, 64:128])
                            nc.vector.tensor_copy(v_sb[t][:, 256:320], pv[:, 128:192])
                            nc.vector.tensor_copy(v_sb[t][:, 448:512], pv[:, 192:256])
                    return fn

                return [piece(lo, lo + 4) for lo in (0, 4)]

            # phase 0 trailing: q for the first tile, v for the first tiles.
            for fn in qproj_pieces(0, 512):
                fn()
            for t in range(2):
                for fn in vproj_pieces(t):
                    fn()

            # ---- attention ----
            ctx_sb = [
                const.tile([128, L], BF16_DT, tag=f"ctx{p}", name=f"ctx_sb{p}")
                for p in range(2)
            ]
            ag_out = {}
            unit_cs = {}

            def normalize(u):
                p, q0, q1 = UNITS[u]
                qsl = slice(q0, q1)
                w = q1 - q0
                cs0, cs1 = unit_cs[u]
                # Copy both PSUM banks to SBUF immediately so the next unit's
                # AV accumulation can reclaim them.
                csA = work.tile([128, 512], BF16_DT, tag="csA", name="csA")
                csB = work.tile([128, 512], BF16_DT, tag="csB", name="csB")
                nc.vector.tensor_copy(csA[:, 0:w], cs0)
                nc.vector.tensor_copy(csB[:, 0:w], cs1)
                rb = ps.tile([128, 512], FP32, tag="op", bufs=2, name="rb")
                nc.tensor.matmul(
                    rb[0:64, 0:w],
                    lhsT=selT[64:128, 0:64],
                    rhs=csA[64:128, 0:w],
                )
                nc.tensor.matmul(
                    rb[64:128, 0:w],
                    lhsT=selT[0:64, 64:128],
                    rhs=csB[0:64, 0:w],
                )
                recipf = work.tile([128, 512], FP32, tag="recipf", name="recipf")
                nc.vector.reciprocal_approx_fast(recipf[:, 0:w], rb[:, 0:w])
                nc.vector.tensor_mul(
                    ctx_sb[p][0:64, qsl], csA[0:64, 0:w], recipf[0:64, 0:w]
                )
                nc.vector.tensor_mul(
                    ctx_sb[p][64:128, qsl], csB[64:128, 0:w], recipf[64:128, 0:w]
                )
                # gathers: q-slices 0-2 do one combined [128,1024] gather
                # (both pairs) after the pair-1 unit; the last slice gathers
                # per pair so the tail only waits on the final 128KB gather.
                gspec = {1: ("c", 0), 3: ("c", 1), 5: ("c", 2),
                         6: ("p0", 3), 7: ("p1", 3)}.get(u)
                if gspec is not None:
                    kind, qi = gspec
                    if kind == "c":
                        ag_in = dram.tile(
                            [128, 1024], BF16_DT, tag=f"agi{qi}", name=f"agi{qi}"
                        )
                        ago = dram.tile(
                            [512, 1024], BF16_DT, tag=f"ago{qi}", name=f"ago{qi}"
                        )
                        ag_out[(qi, 0)] = (ago, 0)
                        ag_out[(qi, 1)] = (ago, 512)
                        nc.sync.dma_start(ag_in[:, 0:512], ctx_sb[0][:, qsl])
                        nc.sync.dma_start(ag_in[:, 512:1024], ctx_sb[1][:, qsl])
                    else:
                        pp = 0 if kind == "p0" else 1
                        ag_in = dram.tile(
                            [128, 512], BF16_DT, tag=f"agi{qi}{pp}",
                            name=f"agi{qi}{pp}"
                        )
                        ago = dram.tile(
                            [512, 512], BF16_DT, tag=f"ago{qi}{pp}",
                            name=f"ago{qi}{pp}"
                        )
                        ag_out[(qi, pp)] = (ago, 0)
                        nc.sync.dma_start(ag_in[:, :], ctx_sb[pp][:, qsl])
                    nc.gpsimd.collective_compute(
                        "AllGather",
                        mybir.AluOpType.bypass,
                        replica_groups=GRPS,
                        ins=[ag_in.opt()],
                        outs=[ago.opt()],
                    )

            def oproj_pieces(q0, q1, src0):
                # output projection for queries [q0, q1); src0 = q-slice index
                # of the combined AllGather output (pair p at col off p*512).
                qsl = slice(q0, q1)
                w = q1 - q0
                state = {}
                srcs = []  # (global kc, pair, rank)
                for pp in range(2):
                    for r in range(4):
                        srcs.append((2 * r + pp, pp, r))

                def load_cf(pp):
                    def fn():
                        ago, off = ag_out[(src0, pp)]
                        for i, (kc, ppi, r) in enumerate(srcs):
                            if ppi != pp:
                                continue
                            t = work.tile(
                                [128, 512], BF16_DT, tag=f"cf{i}", name=f"cf{i}"
                            )
                            nc.gpsimd.dma_start(
                                t[:, 0:w],
                                ago[r * 128 : (r + 1) * 128, off : off + w],
                            )
                            state[i] = t
                    return fn

                def mm_piece(m, lo, hi):
                    def fn():
                        if lo == 0:
                            state[f"po{m}"] = ps.tile(
                                [128, 512], FP32, tag="op", bufs=2, name="o_ps"
                            )
                        po = state[f"po{m}"]
                        for i in range(lo, hi):
                            kc, ppi, r = srcs[i]
                            nc.tensor.matmul(
                                po[:, 0:w],
                                lhsT=wqvo_sb[:, 2 * KC_D + kc, m * 128 : (m + 1) * 128],
                                rhs=state[i][:, 0:w],
                                start=(i == 0),
                                stop=(i == KC_D - 1),
                            )
                        if hi == KC_D:
                            osb = work.tile([128, 512], BF16_DT, tag="osb", name="osb")
                            nc.vector.tensor_scalar_add(
                                osb[:, 0:w], po[:, 0:w], ob_sb[:, m : m + 1]
                            )
                            nc.sync.dma_start(
                                outT[m * 128 : (m + 1) * 128, qsl], osb[:, 0:w]
                            )
                    return fn

                return [
                    load_cf(0),
                    mm_piece(0, 0, 4),
                    mm_piece(1, 0, 4),
                    load_cf(1),
                    mm_piece(0, 4, 8),
                    mm_piece(1, 4, 8),
                ]

            # trailing-work schedule per unit (each accumulation's pieces stay
            # within one unit):
            qp3 = qproj_pieces(1536, 2048)

            def unit_deferred(u):
                if u == 0:
                    out = []
                    for t in range(2, KT):
                        out += vproj_pieces(t)
                    return out
                if u == 1:
                    return qproj_pieces2(0)
                if u == 2:
                    return qproj_pieces2(1)
                if u == 3:
                    return qp3[:4]
                if u == 4:
                    return qp3[4:]
                return []

            for u, (p, q0, q1) in enumerate(UNITS):
                qsl = slice(q0, q1)
                w = q1 - q0
                cs0 = ps.tile([128, 512], FP32, tag="ctx", bufs=1, name="cs0")
                cs1 = ps.tile([128, 512], FP32, tag="sums", bufs=1, name="cs1")
                unit_cs[u] = (cs0[:, 0:w], cs1[:, 0:w])
                deferred = unit_deferred(u)

                def av(kc, pr, p=p, w=w, cs0=cs0, cs1=cs1):
                    st = kc == 0
                    sp = kc == KT - 1
                    nc.tensor.matmul(
                        cs0[:, 0:w],
                        lhsT=v_sb[kc][:, p * 256 : p * 256 + 128],
                        rhs=pr[:, 0:w],
                        start=st,
                        stop=sp,
                    )
                    nc.tensor.matmul(
                        cs1[:, 0:w],
                        lhsT=v_sb[kc][:, p * 256 + 128 : p * 256 + 256],
                        rhs=pr[:, w : 2 * w],
                        start=st,
                        stop=sp,
                    )

                prev = None
                for kc in range(KT):
                    s01 = ps.tile([128, 1024], FP32, tag="sc", bufs=2, name="s01")
                    nc.tensor.matmul(
                        s01[:, 0:w],
                        lhsT=kT2[p][0:64, kc * 128 : (kc + 1) * 128],
                        rhs=qT_sb[p][0:64, qsl],
                    )
                    nc.tensor.matmul(
                        s01[:, w : 2 * w],
                        lhsT=kT2[p][64:128, kc * 128 : (kc + 1) * 128],
                        rhs=qT_sb[p][64:128, qsl],
                    )
                    pr = work.tile([128, 1024], BF16_DT, tag="pr", name="pr")
                    if apply_mask:
                        e01 = work.tile([128, 1024], FP32, tag="e01", name="e01")
                        nc.scalar.activation(e01[:, 0 : 2 * w], s01[:, 0 : 2 * w], Exp)
                        nc.vector.tensor_scalar_mul(
                            pr[:, 0:w], e01[:, 0:w], mask_sb[:, kc : kc + 1]
                        )
                        nc.vector.tensor_scalar_mul(
                            pr[:, w : 2 * w],
                            e01[:, w : 2 * w],
                            mask_sb[:, kc : kc + 1],
                        )
                    else:
                        nc.scalar.activation(pr, s01, Exp)
                    # trailing work keeps the PE busy while exp paces the loop
                    # (unit 0 pops 2/chunk: v projection must stay ~2 tiles
                    # ahead of this unit's own AV consumption; unit 8 pops
                    # late so its pair-1 gather — started at u8's entry — has
                    # landed before the oproj pieces reference it)
                    if deferred and kc >= 1:
                        deferred.pop(0)()
                        if deferred and u == 0:
                            deferred.pop(0)()
                    if prev is not None:
                        av(kc - 1, prev)
                    prev = pr
                av(KT - 1, prev)
                while deferred:
                    deferred.pop(0)()
                normalize(u)

            # tail: the whole output projection.  q0-q2 gathers completed
            # long ago (their cf loads and matmuls never stall); only the q3
            # pieces wait on the final gather.
            for qi in range(4):
                for fn in oproj_pieces(512 * qi, 512 * qi + 512, qi):
                    fn()

    nc.finalize()
    return nc


@functools.lru_cache(maxsize=2)
def _built(apply_mask: bool):
    return _build_nc(apply_mask)


def kernel(input, mask, q_w, q_b, k_w, k_b, v_w, v_b, o_w, o_b):
    global LAST_EXEC_NS, LAST_RESULTS
    input = np.asarray(input, dtype=np.float32)
    mask = np.asarray(mask)
    apply_mask = not bool(np.all(mask != 0))
    nc = _built(apply_mask)

    qw = (np.asarray(q_w, np.float32) / 8.0).astype(BF16)
    kw = np.asarray(k_w, np.float32).astype(BF16)
    vw = np.asarray(v_w, np.float32).astype(BF16)
    ow = np.asarray(o_w, np.float32).astype(BF16)
    qb = np.asarray(q_b, np.float32) / 8.0
    kb = np.asarray(k_b, np.float32)
    ob = (
        np.asarray(o_b, np.float64)
        + np.asarray(o_w, np.float64) @ np.asarray(v_b, np.float64)
    ).astype(np.float32)

    def pack_w(w_ownT):
        # [1024, 256] (transposed weight) -> [128, 8, 256]
        return np.ascontiguousarray(
            w_ownT.reshape(KC_D, 128, OWN).transpose(1, 0, 2)
        )

    in_maps = []
    for c in range(N_CORES):
        b, r = divmod(c, 4)
        own = slice(OWN * r, OWN * (r + 1))
        m01 = (mask[b] != 0).astype(np.float32)
        xT = input[b].T.astype(BF16)  # [1024, 2048]
        in_maps.append(
            {
                "xq": np.ascontiguousarray(
                    xT.reshape(KC_D, 128, L).transpose(1, 0, 2)
                ),
                "wk": pack_w(kw[own, :].T),
                "wqvo": np.ascontiguousarray(
                    np.concatenate(
                        [
                            pack_w(qw[own, :].T),
                            pack_w(vw[own, :].T),
                            pack_w(ow[own, :].T),
                        ],
                        axis=1,
                    )
                ),
                "qb2": np.ascontiguousarray(qb[own].reshape(2, 128).T),
                "kb2": np.ascontiguousarray(kb[own].reshape(2, 128).T),
                "ob2": np.ascontiguousarray(ob[own].reshape(2, 128).T),
                "mask01": np.ascontiguousarray(m01.reshape(KT, 128).T),
            }
        )

    trace = os.environ.get("KERNEL_TRACE", "0") == "1"
    res = run_bass_kernel_spmd(
        nc,
        in_maps,
        core_ids=list(range(N_CORES)),
        trace=trace,
        trace_cores=list(range(N_CORES)) if trace else None,
        stitch_traces=False,
    )
    LAST_EXEC_NS = res.exec_time_ns
    LAST_RESULTS = res

    out = np.empty((BS, L, D), dtype=np.float32)
    for c in range(N_CORES):
        b, r = divmod(c, 4)
        out[b, :, OWN * r : OWN * (r + 1)] = res.results[c]["outT"].T.astype(
            np.float32
        )
    return out
